# revision 4
# baseline (speedup 1.0000x reference)
"""Mixtral decoder layer (GQA attention + top2-of-28-combination MoE) on 8 TRN2 cores.

v2 SPMD design (one program; per-core behavior injected via inputs):
  - Attention head-sharded: core c owns q-heads {2c,2c+1} / kv-head c over ALL
    tokens. Both q-heads share kv-head c, so scores/exp/ctx run on combined
    [keys=128, 2*256] tiles (half the instructions of v1). RoPE folded into a
    rotated-weight set. Softmax denominator via ones column appended to V.
    All activations bf16 (fp32 PSUM accumulation).
  - Context re-sharded token-wise via AllToAll; O-proj + residual + rmsnorm2 +
    router + top2-of-28 routing per 256-token zigzag block {c, 15-c} (f32).
  - MoE expert-parallel with A2A dispatch/combine: owner-local routing computes
    per-(expert, owner) slot positions (capacity CAPP); x2 rows scattered into
    a [E*CAPP, H] send buffer; AllToAll delivers each expert-core its tokens
    already compacted; FFN (bf16, fp32 accum, Silu); results transposed back to
    natural rows and AllToAll'd back; owners gather their two expert rows by
    the same offsets and combine with router weights + residual.
  - ln1/ln2 and 1/sqrt(HD) folded into weights host-side.
"""

import itertools

import numpy as np

import concourse.bass as bass
import concourse.tile as tile
from concourse import bacc, bass_utils, mybir

P = 128
B, S, H = 1, 2048, 1024
NH, KVH, HD = 16, 8, 64
E, TOPK, I = 8, 2, 3584
EPS = 1e-6
THETA = 1000000.0
NCORES = 8
NT = S // P
NPAIR = NT // 2
NI = I // P
CAPP = 112                  # per-(expert, owner) slot capacity
SLOTS = E * CAPP            # 896 rows through the dispatch/return A2As
NST = SLOTS // P            # 7 slot tiles
NEG = -1.0e30

f32 = mybir.dt.float32
f32r = mybir.dt.float32r
bf16 = mybir.dt.bfloat16
i32 = mybir.dt.int32
BF16_NP = mybir.dt.np(bf16)

COMBS = np.array(list(itertools.combinations(range(E), TOPK)), dtype=np.int64)

AluOp = mybir.AluOpType
Act = mybir.ActivationFunctionType
AxX = mybir.AxisListType.X

PHASE_MARKS = []


def _mark(nc, name):
    PHASE_MARKS.append((name, nc.next_id()))


def _z_a_of_block(b):
    return (b, 0) if b < 8 else (15 - b, 1)


def build_program(dbg: bool = False, n_unroll: int = 1, skip=()):
    """skip: subset of {"attn","ffn","coll","front"} for timing bisection."""
    nc = bacc.Bacc("TRN2", target_bir_lowering=False, debug=False,
                   num_devices=NCORES)

    def din(name, shape, dtype=f32):
        return nc.dram_tensor(name, list(shape), dtype, kind="ExternalInput").ap()

    t = {}
    t["hT_full"] = din("hT_full", [H, S])
    t["hT_my"] = din("hT_my", [H, 256])
    t["wqkvT"] = din("wqkvT", [H, 448])
    t["woT"] = din("woT", [P, 8 * H], bf16)          # [p, dt*H+j]
    t["gateT"] = din("gateT", [P, 8 * E])            # [p, kt*E+e]
    t["cosT"] = din("cosT", [P, S])
    t["sinT"] = din("sinT", [P, S])
    t["ident"] = din("ident", [P, P])
    t["tri"] = din("tri", [P, P])
    t["onescol"] = din("onescol", [P, 1])
    t["onescol_b"] = din("onescol_b", [P, 1], bf16)
    t["epscol"] = din("epscol", [P, 1])
    t["onescol_r"] = din("onescol_r", [P, 1], f32r)
    t["ones1_r"] = din("ones1_r", [1, P], f32r)
    t["mcomb_r"] = din("mcomb_r", [E, 28], f32r)
    t["selmat_r"] = din("selmat_r", [28, E], f32r)
    t["cumtri_r"] = din("cumtri_r", [E, E], f32r)    # [e',e]=1 iff e'<=e
    t["ecapp"] = din("ecapp", [E, 1])                # e*CAPP - 1
    t["zrow"] = din("zrow", [P, 2048], bf16)
    t["w13"] = din("w13", [NI, P, 2048], bf16)       # [it, p, kt*256+j]
    t["w2c"] = din("w2c", [P, NI * H], bf16)         # [p, it*H+h]

    def dout(name, shape, dtype=f32):
        return nc.dram_tensor(name, list(shape), dtype, kind="ExternalOutput").ap()

    t["out_blk"] = dout("out_blk", [256, H])
    t["dbg"] = {}
    if dbg:
        for nm, shp, dt_ in [
            ("d_x1T", [P, 512], f32), ("d_qr", [64, 1024], bf16),
            ("d_raw", [P, 8, 512], f32), ("d_ssq", [4, 512], f32),
            ("d_xn", [P, 8, 512], f32), ("d_qps", [P, 512], f32),
            ("d_kr", [64, S], bf16), ("d_ctxT", [P, S], bf16),
            ("d_h2T", [H, 256], f32), ("d_rt", [16, 256], f32),
            ("d_off", [P, 8], i32), ("d_xg", [P, SLOTS], bf16),
            ("d_hp", [P, SLOTS], bf16), ("d_y", [P, 2 * H], bf16),
        ]:
            t["dbg"][nm] = dout(nm, shp, dt_)

    PHASE_MARKS.clear()
    rg = [list(range(NCORES))]
    with tile.TileContext(nc) as tc:
        for rep in range(n_unroll):
            _emit_once(nc, tc, rg, t, rep, skip)
    nc.compile()
    return nc


def _emit_once(nc, tc, rg, t, rep, skip=()):
    _mark(nc, "start")
    dbg = t["dbg"] if rep == 0 else {}
    r = f"r{rep}_"

    with nc.allow_low_precision(reason="bf16 compute with f32 psum accum"), \
         tc.tile_pool(name=r + "const", bufs=1) as cpool, \
         tc.tile_pool(name=r + "big", bufs=1) as big, \
         tc.tile_pool(name=r + "dram", bufs=1, space="DRAM") as dram:

        _mark(nc, "consts")
        ident_sb = cpool.tile([P, P], f32)
        nc.sync.dma_start(ident_sb[:], t["ident"])
        ident_r_sb = cpool.tile([P, P], f32r)
        nc.sync.dma_start(ident_r_sb[:], t["ident"].bitcast(f32r))
        ident_b_sb = cpool.tile([P, P], bf16)
        nc.vector.tensor_copy(ident_b_sb[:], ident_sb[:])
        tri_sb = cpool.tile([P, P], f32)
        nc.sync.dma_start(tri_sb[:], t["tri"])
        onescol_sb = cpool.tile([P, 1], f32)
        nc.sync.dma_start(onescol_sb[:], t["onescol"])
        onescol_b_sb = cpool.tile([P, 1], bf16)
        nc.sync.dma_start(onescol_b_sb[:], t["onescol_b"])
        epscol_sb = cpool.tile([P, 1], f32)
        nc.sync.dma_start(epscol_sb[:], t["epscol"])
        onescol_r_sb = cpool.tile([P, 1], f32r)
        nc.sync.dma_start(onescol_r_sb[:], t["onescol_r"])
        ones1_r_sb = cpool.tile([1, P], f32r)
        nc.sync.dma_start(ones1_r_sb[:], t["ones1_r"])
        mcomb_sb = cpool.tile([E, 28], f32r)
        nc.sync.dma_start(mcomb_sb[:], t["mcomb_r"])
        selmat_sb = cpool.tile([28, E], f32r)
        nc.sync.dma_start(selmat_sb[:], t["selmat_r"])
        cumtri_sb = cpool.tile([E, E], f32r)
        nc.sync.dma_start(cumtri_sb[:], t["cumtri_r"])
        ecapp_sb = cpool.tile([E, 1], f32)
        nc.sync.dma_start(ecapp_sb[:], t["ecapp"])
        zrow_sb = cpool.tile([P, 2048], bf16)
        if skip:
            nc.sync.dma_start(zrow_sb[:], t["zrow"])

        _mark(nc, "load_h")
        hmy_sb = big.tile([P, 8, 256], f32)
        h2_sb = big.tile([P, 8, 256], f32)
        # dispatch offsets/weights, persist till combine: cols (tt*2+k)
        offc_sb = big.tile([P, 4], i32)
        wc_sb = big.tile([P, 4], f32)

        # ---- internal DRAM ----
        a2a_in = dram.tile([NCORES * P, 256], bf16)
        a2a_out = dram.tile([NCORES * P, 256], bf16)
        disp_in = dram.tile([SLOTS, H], bf16)
        disp_out = dram.tile([SLOTS, H], bf16)
        ret_in = dram.tile([SLOTS, H], bf16)
        ret_out = dram.tile([SLOTS, H], bf16)

        # ================= attention scope =================
        with tc.tile_pool(name=r + "attn", bufs=1) as apool, \
             tc.tile_pool(name=r + "aw", bufs=2) as aw:

            wqkv_sb = apool.tile([P, 8, 448], f32r)
            nc.sync.dma_start(wqkv_sb[:],
                              t["wqkvT"].bitcast(f32r).rearrange("(kt p) n -> p kt n", p=P))
            qr01_sb = apool.tile([64, NPAIR, 2, 256], bf16)
            kr_sb = apool.tile([64, S], bf16)
            vb_sb = apool.tile([P, NT, 65], bf16)
            a2a_sb = apool.tile([P, NCORES, 256], bf16)

            _mark(nc, "qkv_rope")
            rmsps_cm = tc.tile_pool(name=r + "rmsps", bufs=1, space="PSUM")
            rmsps = rmsps_cm.__enter__()
            qkvps_cm = tc.tile_pool(name=r + "qkvps", bufs=1, space="PSUM")
            qkvps = qkvps_cm.__enter__()
            qkvps2_cm = tc.tile_pool(name=r + "qkvps2", bufs=2, space="PSUM")
            qkvps2 = qkvps2_cm.__enter__()
            x1_sb = apool.tile([P, 8, S], f32r)    # normalized in place
            nc.sync.dma_start(x1_sb[:],
                              t["hT_full"].bitcast(f32r).rearrange("(kt p) n -> p kt n", p=P))
            cos_sb = apool.tile([P, S], f32)
            nc.sync.dma_start(cos_sb[:], t["cosT"])
            sin_sb = apool.tile([P, S], f32)
            nc.sync.dma_start(sin_sb[:], t["sinT"])
            for ntile in range(4):
                nsl = slice(ntile * 512, (ntile + 1) * 512)
                ssq = rmsps.tile([1, 512], f32, tag="ssq")
                for kt in range(8):
                    xsq = aw.tile([P, 512], f32r, tag="xsq")
                    nc.scalar.activation(xsq[:], x1_sb[:, kt, nsl], Act.Square)
                    nc.tensor.matmul(ssq[:], onescol_r_sb[:], xsq[:],
                                     start=(kt == 0), stop=(kt == 7))
                srow = aw.tile([1, 512], f32, tag="srow")
                nc.scalar.activation(srow[:], ssq[:], Act.Sqrt,
                                     bias=epscol_sb[0:1, :], scale=1.0 / H)
                srow_r = aw.tile([1, 512], f32r, tag="srow_r")
                nc.vector.reciprocal(srow_r[:], srow[:])
                sbc = rmsps.tile([P, 512], f32, tag="sbc")
                nc.tensor.matmul(sbc[:], ones1_r_sb[:1, :], srow_r[:],
                                 start=True, stop=True)
                for kt in range(8):
                    nc.vector.tensor_mul(x1_sb[:, kt, nsl], x1_sb[:, kt, nsl], sbc[:])
                # QKV + RoPE
                q_ps = qkvps.tile([P, 512], f32, tag="q_ps")
                qR_ps = qkvps.tile([P, 512], f32, tag="qR_ps")
                kk_ps = qkvps.tile([P, 512], f32, tag="kk_ps")
                v_ps = qkvps.tile([64, 512], f32, tag="v_ps")
                for kt in range(8):
                    x1s = x1_sb[:, kt, nsl]
                    st, sp = kt == 0, kt == 7
                    nc.tensor.matmul(q_ps[:], wqkv_sb[:, kt, 0:128], x1s, start=st, stop=sp)
                    nc.tensor.matmul(qR_ps[:], wqkv_sb[:, kt, 128:256], x1s, start=st, stop=sp)
                    nc.tensor.matmul(kk_ps[:], wqkv_sb[:, kt, 256:384], x1s, start=st, stop=sp)
                    nc.tensor.matmul(v_ps[:], wqkv_sb[:, kt, 384:448], x1s, start=st, stop=sp)
                t1 = aw.tile([P, 512], f32, tag="rope1")
                t2 = aw.tile([P, 512], f32, tag="rope2")
                nc.vector.tensor_mul(t1[:], q_ps[:], cos_sb[:, nsl])
                nc.vector.tensor_mul(t2[:], qR_ps[:], sin_sb[:, nsl])
                for j in range(2):
                    pr = 2 * ntile + j
                    csl = slice(j * 256, (j + 1) * 256)
                    nc.vector.tensor_add(qr01_sb[:, pr, 0, :], t1[0:64, csl], t2[0:64, csl])
                    nc.vector.tensor_add(qr01_sb[:, pr, 1, :], t1[64:128, csl], t2[64:128, csl])
                nc.vector.tensor_mul(t1[:64, :], kk_ps[0:64, :], cos_sb[0:64, nsl])
                nc.vector.tensor_mul(t2[:64, :], kk_ps[64:128, :], sin_sb[0:64, nsl])
                nc.vector.tensor_add(kr_sb[:, nsl], t1[:64, :], t2[:64, :])
                v_sb = aw.tile([64, 512], f32, tag="v_sb")
                nc.vector.tensor_copy(v_sb[:], v_ps[:])
                for tt in range(4):
                    ti = ntile * 4 + tt
                    vtp = qkvps2.tile([P, 64], f32, tag="vtp")
                    nc.tensor.transpose(vtp[:], v_sb[:, tt * 128:(tt + 1) * 128],
                                        ident_sb[:64, :64])
                    nc.vector.tensor_copy(vb_sb[:, ti, 0:64], vtp[:])
            nc.vector.tensor_copy(vb_sb[:, :, 64:65],
                                  onescol_b_sb[:].to_broadcast([P, NT, 1]))
            qkvps2_cm.__exit__(None, None, None)
            qkvps_cm.__exit__(None, None, None)
            rmsps_cm.__exit__(None, None, None)
            if dbg:
                nc.sync.dma_start(dbg["d_qr"],
                                  qr01_sb[:, 0:2, :, :].rearrange("p a b n -> p (a b n)"))
                nc.sync.dma_start(dbg["d_kr"], kr_sb[:])

            _mark(nc, "attn_core")
            attps_cm = tc.tile_pool(name=r + "attps", bufs=2, space="PSUM")
            attps = attps_cm.__enter__()
            attpsA_cm = tc.tile_pool(name=r + "attpsA", bufs=2, space="PSUM")
            attpsA = attpsA_cm.__enter__()
            for pr in ([] if "attn" in skip else range(NPAIR)):
                ctx = attpsA.tile([65, 512], f32, tag="ctx")
                for si in range(2 * pr + 2):
                    st_ps = attps.tile([P, 512], f32, tag="st_ps")
                    nc.tensor.matmul(st_ps[:], kr_sb[:, si * 128:(si + 1) * 128],
                                     qr01_sb[:, pr, :, :].rearrange("p a n -> p (a n)"),
                                     start=True, stop=True)
                    if si == 2 * pr:
                        nc.vector.tensor_add(st_ps[:, 0:128], st_ps[:, 0:128], tri_sb[:])
                        nc.vector.tensor_add(st_ps[:, 256:384], st_ps[:, 256:384], tri_sb[:])
                    if si == 2 * pr + 1:
                        nc.vector.tensor_add(st_ps[:, 128:256], st_ps[:, 128:256], tri_sb[:])
                        nc.vector.tensor_add(st_ps[:, 384:512], st_ps[:, 384:512], tri_sb[:])
                        nc.vector.tensor_scalar(st_ps[:, 0:128], st_ps[:, 0:128],
                                                NEG, None, op0=AluOp.add)
                        nc.vector.tensor_scalar(st_ps[:, 256:384], st_ps[:, 256:384],
                                                NEG, None, op0=AluOp.add)
                    stexp = aw.tile([P, 512], bf16, tag="stexp")
                    nc.scalar.activation(stexp[:], st_ps[:], Act.Exp)
                    nc.tensor.matmul(ctx[:], vb_sb[:, si, :], stexp[:],
                                     start=(si == 0), stop=(si == 2 * pr + 1))
                rec = aw.tile([1, 512], f32r, tag="rec")
                nc.vector.reciprocal(rec[:], ctx[64:65, :])
                dbc = attps.tile([64, 512], f32, tag="dbc")
                nc.tensor.matmul(dbc[:], ones1_r_sb[:1, 0:64], rec[:],
                                 start=True, stop=True)
                dbc_sb = aw.tile([64, 512], f32, tag="dbc_sb")
                nc.scalar.activation(dbc_sb[:], dbc[:], Act.Copy)
                for h in range(2):
                    for a in range(2):
                        ti = 2 * pr + a
                        z, az = _z_a_of_block(ti)
                        csl = slice(h * 256 + a * 128, h * 256 + a * 128 + 128)
                        nc.vector.tensor_mul(
                            a2a_sb[h * 64:h * 64 + 64, z, az * 128:az * 128 + 128],
                            ctx[0:64, csl], dbc_sb[:, csl])
            attpsA_cm.__exit__(None, None, None)
            attps_cm.__exit__(None, None, None)
            if "attn" in skip:
                nc.vector.tensor_copy(a2a_sb[:].rearrange("p c n -> p (c n)"),
                                      zrow_sb[:, 0:2048])
            _mark(nc, "a2a")
            nc.sync.dma_start(a2a_in[:].rearrange("(c p) n -> p c n", p=P), a2a_sb[:])
            if dbg:
                nc.sync.dma_start(dbg["d_ctxT"],
                                  a2a_sb[:].rearrange("p c n -> p (c n)"))

        if "coll" in skip:
            nc.sync.dma_start(a2a_out[:], a2a_in[:])
        else:
            nc.gpsimd.collective_compute(
                "AllToAll", AluOp.bypass, replica_groups=rg,
                ins=[a2a_in.opt()], outs=[a2a_out.opt()])

        # ================= O-proj + norm2 + router + dispatch =================
        nc.sync.dma_start(hmy_sb[:], t["hT_my"].rearrange("(kt p) n -> p kt n", p=P))
        _mark(nc, "oproj")
        with tc.tile_pool(name=r + "oproj", bufs=1) as opool, \
             tc.tile_pool(name=r + "ow", bufs=2) as ow:

            x2_sb = opool.tile([P, 8, 256], f32r)
            ctxmy_sb = opool.tile([P, 8, 256], bf16)
            nc.sync.dma_start(ctxmy_sb[:], a2a_out[:].rearrange("(c p) n -> p c n", p=P))
            if dbg:
                dcm = opool.tile([P, 8, 256], f32)
                for kt in range(8):
                    nc.vector.tensor_copy(dcm[:, kt, :], ctxmy_sb[:, kt, :])
                nc.sync.dma_start(dbg["d_raw"][:, :, 0:256], dcm[:])
            wo_sb = opool.tile([P, 8, H], bf16)
            nc.sync.dma_start(wo_sb[:], t["woT"].rearrange("p (dt n) -> p dt n", dt=8))
            o1_cm = tc.tile_pool(name=r + "o1", bufs=2, space="PSUM")
            o1 = o1_cm.__enter__()
            for hd in range(8):
                o_ps = o1.tile([P, 256], f32, tag="o_ps")
                for dt_ in range(8):
                    nc.tensor.matmul(o_ps[:], wo_sb[:, dt_, hd * 128:(hd + 1) * 128],
                                     ctxmy_sb[:, dt_, :], start=(dt_ == 0), stop=(dt_ == 7))
                nc.vector.tensor_add(h2_sb[:, hd, :], o_ps[:], hmy_sb[:, hd, :])
            if dbg:
                nc.sync.dma_start(dbg["d_h2T"].rearrange("(kt p) n -> p kt n", p=P), h2_sb[:])
            o1_cm.__exit__(None, None, None)

            _mark(nc, "rmsnorm2")
            o2_cm = tc.tile_pool(name=r + "o2", bufs=1, space="PSUM")
            o2 = o2_cm.__enter__()
            ssq2 = o2.tile([1, 256], f32, tag="ssq2")
            for kt in range(8):
                xsq2 = ow.tile([P, 256], f32r, tag="xsq2")
                nc.scalar.activation(xsq2[:], h2_sb[:, kt, :], Act.Square)
                nc.tensor.matmul(ssq2[:], onescol_r_sb[:], xsq2[:],
                                 start=(kt == 0), stop=(kt == 7))
            srow2 = ow.tile([1, 256], f32, tag="srow2")
            nc.scalar.activation(srow2[:], ssq2[:], Act.Sqrt,
                                 bias=epscol_sb[0:1, :], scale=1.0 / H)
            srow2_r = ow.tile([1, 256], f32r, tag="srow2_r")
            nc.vector.reciprocal(srow2_r[:], srow2[:])
            sbc2 = o2.tile([P, 256], f32, tag="sbc2")
            nc.tensor.matmul(sbc2[:], ones1_r_sb[:1, :], srow2_r[:], start=True, stop=True)
            for kt in range(8):
                nc.vector.tensor_mul(x2_sb[:, kt, :], h2_sb[:, kt, :], sbc2[:])
            o2_cm.__exit__(None, None, None)

            _mark(nc, "router")
            o3_cm = tc.tile_pool(name=r + "o3", bufs=1, space="PSUM")
            o3 = o3_cm.__enter__()
            gate_sb = opool.tile([P, 8, E], f32r)
            nc.sync.dma_start(gate_sb[:],
                              t["gateT"].bitcast(f32r).rearrange("p (kt n) -> p kt n", kt=8))
            rw_sb = ow.tile([P, 2, E], f32r, tag="rw")
            for tt in range(2):
                lg_ps = o3.tile([P, E], f32, tag="lg_ps")
                for kt in range(8):
                    nc.tensor.matmul(lg_ps[:], x2_sb[:, kt, tt * 128:(tt + 1) * 128],
                                     gate_sb[:, kt, :], start=(kt == 0), stop=(kt == 7))
                mx = ow.tile([P, 1], f32, tag="mx")
                nc.vector.tensor_reduce(mx[:], lg_ps[:], axis=AxX, op=AluOp.max)
                mxn = ow.tile([P, 1], f32, tag="mxn")
                nc.vector.tensor_scalar_mul(mxn[:], mx[:], -1.0)
                ex = ow.tile([P, E], f32, tag="ex")
                sm = ow.tile([P, 1], f32, tag="sm")
                nc.scalar.activation(ex[:], lg_ps[:], Act.Exp, bias=mxn[:], accum_out=sm[:])
                smr = ow.tile([P, 1], f32, tag="smr")
                nc.vector.reciprocal(smr[:], sm[:])
                nc.vector.tensor_scalar(rw_sb[:, tt, :], ex[:], smr[:], None, op0=AluOp.mult)
            rwT_sb = ow.tile([E, 256], f32r, tag="rwT")
            for tt in range(2):
                rwt_ps = o3.tile([E, P], f32r, tag="rwt_ps")
                nc.tensor.transpose(rwt_ps[:], rw_sb[:, tt, :], ident_r_sb[:])
                nc.vector.tensor_copy(rwT_sb[:, tt * 128:(tt + 1) * 128], rwt_ps[:])
            mask_sb = ow.tile([P, 2, 28], f32r, tag="mask")
            for tt in range(2):
                cb_ps = o3.tile([P, 28], f32, tag="cb_ps")
                nc.tensor.matmul(cb_ps[:], rwT_sb[:, tt * 128:(tt + 1) * 128],
                                 mcomb_sb[:], start=True, stop=True)
                mxc = ow.tile([P, 1], f32, tag="mxc")
                nc.vector.tensor_reduce(mxc[:], cb_ps[:], axis=AxX, op=AluOp.max)
                nc.vector.tensor_scalar(mask_sb[:, tt, :], cb_ps[:], mxc[:], None,
                                        op0=AluOp.is_ge)
            selT_sb = ow.tile([E, 256], f32r, tag="selT")
            for tt in range(2):
                mkt_ps = o3.tile([28, P], f32r, tag="mkt_ps")
                nc.tensor.transpose(mkt_ps[:], mask_sb[:, tt, :], ident_r_sb[:])
                mkt = ow.tile([28, P], f32r, tag="mkt")
                nc.vector.tensor_copy(mkt[:], mkt_ps[:])
                selq = o3.tile([E, P], f32, tag="selq")
                nc.tensor.matmul(selq[:], selmat_sb[:], mkt[:], start=True, stop=True)
                nc.vector.tensor_copy(selT_sb[:, tt * 128:(tt + 1) * 128], selq[:])
            # normalized top2 weights
            rwsel_sb = ow.tile([E, 256], f32r, tag="rwsel")
            nc.vector.tensor_mul(rwsel_sb[:], rwT_sb[:], selT_sb[:])
            nrm_ps = o3.tile([1, 256], f32, tag="nrm_ps")
            nc.tensor.matmul(nrm_ps[:], onescol_r_sb[:E, :], rwsel_sb[:],
                             start=True, stop=True)
            nrmr = ow.tile([1, 256], f32r, tag="nrmr")
            nc.vector.reciprocal(nrmr[:], nrm_ps[:])
            nbc_ps = o3.tile([E, 256], f32, tag="nbc_ps")
            nc.tensor.matmul(nbc_ps[:], ones1_r_sb[:1, :E], nrmr[:], start=True, stop=True)
            wsel_sb = ow.tile([E, 256], f32r, tag="wsel")
            nc.vector.tensor_mul(wsel_sb[:], rwsel_sb[:], nbc_ps[:])
            o3_cm.__exit__(None, None, None)

            _mark(nc, "routing")
            # first/second expert split + slot positions
            o5_cm = tc.tile_pool(name=r + "o5", bufs=1, space="PSUM")
            o5 = o5_cm.__enter__()
            cum_ps = o5.tile([E, 256], f32, tag="cum_ps")
            nc.tensor.matmul(cum_ps[:], cumtri_sb[:], selT_sb[:], start=True, stop=True)
            mle1 = ow.tile([E, 256], f32r, tag="mle1")
            nc.vector.tensor_scalar(mle1[:], cum_ps[:], 1.5, None, op0=AluOp.is_le)
            sel1 = ow.tile([E, 256], f32r, tag="sel1")
            nc.vector.tensor_mul(sel1[:], selT_sb[:], mle1[:])
            sel2 = ow.tile([E, 256], f32r, tag="sel2")
            nc.vector.tensor_sub(sel2[:], selT_sb[:], sel1[:])
            pos = ow.tile([E, 256], f32, tag="pos")
            nc.vector.tensor_tensor_scan(pos[:], selT_sb[:], selT_sb[:], 0.0,
                                         op0=AluOp.add, op1=AluOp.bypass)
            offa = ow.tile([E, 256], f32r, tag="offa")
            nc.vector.tensor_scalar(offa[:], pos[:], ecapp_sb[:], None, op0=AluOp.add)
            ovf = ow.tile([E, 256], f32r, tag="ovf")
            nc.vector.tensor_scalar(ovf[:], pos[:], CAPP + 0.5, None, op0=AluOp.is_ge)
            nc.vector.tensor_scalar(ovf[:], ovf[:], 8192.0, None, op0=AluOp.mult)
            nc.vector.tensor_add(offa[:], offa[:], ovf[:])
            # reduce to per-token rows: [1, 4*256] = [off1 | off2 | w1 | w2]
            tmp4 = ow.tile([E, 4, 256], f32r, tag="tmp4")
            for j, (selx, val) in enumerate(((sel1, offa), (sel2, offa),
                                             (sel1, wsel_sb), (sel2, wsel_sb))):
                nc.vector.tensor_mul(tmp4[:, j, :], selx[:], val[:])
            rows_ps = o5.tile([1, 4 * 256], f32, tag="rows_ps")
            tmp4f = tmp4[:].rearrange("p a n -> p (a n)")
            nc.tensor.matmul(rows_ps[:, 0:512], onescol_r_sb[:E, :],
                             tmp4f[:, 0:512], start=True, stop=True)
            nc.tensor.matmul(rows_ps[:, 512:1024], onescol_r_sb[:E, :],
                             tmp4f[:, 512:1024], start=True, stop=True)
            rows_sb = ow.tile([1, 4 * 256], f32, tag="rows_sb")
            nc.vector.tensor_copy(rows_sb[:], rows_ps[:])
            if dbg:
                nc.sync.dma_start(dbg["d_rt"][0:1, :],
                                  rows_sb[:, 0:256])
                nc.sync.dma_start(dbg["d_rt"][1:2, :], rows_sb[:, 256:512])
                nc.sync.dma_start(dbg["d_rt"][2:3, :], rows_sb[:, 512:768])
                nc.sync.dma_start(dbg["d_rt"][3:4, :], rows_sb[:, 768:1024])
            for tt in range(2):
                for j in range(2):  # off1, off2
                    cps = o5.tile([P, 1], f32, tag="cps")
                    nc.tensor.transpose(
                        cps[:], rows_sb[:, j * 256 + tt * 128:j * 256 + (tt + 1) * 128],
                        ident_sb[:1, :1])
                    nc.vector.tensor_copy(offc_sb[:, tt * 2 + j:tt * 2 + j + 1], cps[:])
                for j in range(2):  # w1, w2
                    cps = o5.tile([P, 1], f32, tag="cps")
                    nc.tensor.transpose(
                        cps[:], rows_sb[:, (2 + j) * 256 + tt * 128:(2 + j) * 256 + (tt + 1) * 128],
                        ident_sb[:1, :1])
                    nc.vector.tensor_copy(wc_sb[:, tt * 2 + j:tt * 2 + j + 1], cps[:])
            o5_cm.__exit__(None, None, None)

            _mark(nc, "x2nat")
            o4_cm = tc.tile_pool(name=r + "o4", bufs=2, space="PSUM")
            o4 = o4_cm.__enter__()
            x2n_sb = opool.tile([P, 2, H], bf16)
            for tt in range(2):
                for kt in range(8):
                    xt_ps = o4.tile([P, P], f32r, tag="xt_ps")
                    nc.tensor.transpose(xt_ps[:], x2_sb[:, kt, tt * 128:(tt + 1) * 128],
                                        ident_r_sb[:])
                    nc.vector.tensor_copy(x2n_sb[:, tt, kt * 128:(kt + 1) * 128], xt_ps[:])
            o4_cm.__exit__(None, None, None)

            _mark(nc, "dispatch")
            for tt in range(2):
                for j in ([] if "front" in skip else range(2)):
                    nc.gpsimd.indirect_dma_start(
                        out=disp_in[:],
                        out_offset=bass.IndirectOffsetOnAxis(
                            ap=offc_sb[:, tt * 2 + j:tt * 2 + j + 1], axis=0),
                        in_=x2n_sb[:, tt, :], in_offset=None,
                        bounds_check=SLOTS - 1, oob_is_err=False)
            if dbg:
                nc.sync.dma_start(dbg["d_off"][:, 0:4], offc_sb[:])
                nc.sync.dma_start(dbg["d_off"][:, 4:8], wc_sb[:].bitcast(i32))

        if "coll" in skip:
            nc.sync.dma_start(disp_out[:], disp_in[:])
        else:
            nc.gpsimd.collective_compute(
                "AllToAll", AluOp.bypass, replica_groups=rg,
                ins=[disp_in.opt()], outs=[disp_out.opt()])

        # ================= MoE FFN scope =================
        with tc.tile_pool(name=r + "moe", bufs=1) as mpool, \
             tc.tile_pool(name=r + "mw", bufs=2) as mw:

            _mark(nc, "gather")
            xg_sb = mpool.tile([P, 8, SLOTS], bf16)
            mx_cm = tc.tile_pool(name=r + "mx", bufs=1)
            mxp = mx_cm.__enter__()
            xin_sb = mxp.tile([P, NST, H], bf16)
            nc.sync.dma_start(xin_sb[:], disp_out[:].rearrange("(a p) n -> p a n", p=P))
            m2_cm = tc.tile_pool(name=r + "m2", bufs=2, space="PSUM")
            m2 = m2_cm.__enter__()
            for kt in range(8):
                for a in range(NST):
                    gt_ps = m2.tile([P, P], bf16, tag="gt_ps")
                    nc.tensor.transpose(gt_ps[:], xin_sb[:, a, kt * 128:(kt + 1) * 128],
                                        ident_b_sb[:])
                    nc.vector.tensor_copy(xg_sb[:, kt, a * 128:(a + 1) * 128], gt_ps[:])
            m2_cm.__exit__(None, None, None)
            mx_cm.__exit__(None, None, None)
            if dbg:
                nc.sync.dma_start(dbg["d_xg"], xg_sb[:, 0, :])

            _mark(nc, "ffnA")
            m3_cm = tc.tile_pool(name=r + "m3", bufs=2, space="PSUM")
            m3 = m3_cm.__enter__()
            hp_sb = mpool.tile([P, NI, SLOTS], bf16)
            for it in ([] if "ffn" in skip else range(NI)):
                w13_sb = mw.tile([P, 8, 256], bf16, tag="w13_sb")
                nc.sync.dma_start(w13_sb[:],
                                  t["w13"][it].rearrange("p (kt n) -> p kt n", kt=8))
                h1 = m3.tile([P, SLOTS], f32, tag="h1")
                h3 = m3.tile([P, SLOTS], f32, tag="h3")
                for kt in range(8):
                    st, sp = kt == 0, kt == 7
                    nc.tensor.matmul(h1[:, 0:512], w13_sb[:, kt, 0:128],
                                     xg_sb[:, kt, 0:512], start=st, stop=sp)
                    nc.tensor.matmul(h1[:, 512:896], w13_sb[:, kt, 0:128],
                                     xg_sb[:, kt, 512:896], start=st, stop=sp)
                    nc.tensor.matmul(h3[:, 0:512], w13_sb[:, kt, 128:256],
                                     xg_sb[:, kt, 0:512], start=st, stop=sp)
                    nc.tensor.matmul(h3[:, 512:896], w13_sb[:, kt, 128:256],
                                     xg_sb[:, kt, 512:896], start=st, stop=sp)
                sg = mw.tile([P, SLOTS], f32, tag="sg")
                nc.scalar.activation(sg[:], h1[:], Act.Sigmoid)
                hp1 = mw.tile([P, SLOTS], f32, tag="hp1")
                nc.vector.tensor_mul(hp1[:], h1[:], sg[:])
                nc.vector.tensor_mul(hp_sb[:, it, :], hp1[:], h3[:])
            m3_cm.__exit__(None, None, None)
            if dbg:
                nc.sync.dma_start(dbg["d_hp"], hp_sb[:, 0, :])

            _mark(nc, "ffnB")
            mo_sb = mpool.tile([P, 8, SLOTS], bf16)
            for cc, (c0, cw) in ([] if "ffn" in skip else list(enumerate(((0, 448), (448, 448))))):
                m4_cm = tc.tile_pool(name=r + f"m4_{cc}", bufs=1, space="PSUM")
                m4 = m4_cm.__enter__()
                mo_ps = []
                for hd in range(8):
                    mo_hd = m4.tile([P, cw], f32, tag=f"mo{hd}")
                    mo_ps.append(mo_hd)
                for it in range(NI):
                    w2_sb = mw.tile([P, H], bf16, tag="w2_sb")
                    nc.sync.dma_start(
                        w2_sb[:],
                        t["w2c"].rearrange("p (it n) -> p it n", it=NI)[:, it, :])
                    for hd in range(8):
                        nc.tensor.matmul(mo_ps[hd][:], w2_sb[:, hd * 128:(hd + 1) * 128],
                                         hp_sb[:, it, c0:c0 + cw],
                                         start=(it == 0), stop=(it == NI - 1))
                for hd in range(8):
                    nc.vector.tensor_copy(mo_sb[:, hd, c0:c0 + cw], mo_ps[hd][:])
                m4_cm.__exit__(None, None, None)

            _mark(nc, "retT")
            ret_nat = mpool.tile([P, NST, H], bf16)
            m5_cm = tc.tile_pool(name=r + "m5", bufs=2, space="PSUM")
            m5 = m5_cm.__enter__()
            for a in ([] if "ffn" in skip else range(NST)):
                for hd in range(8):
                    rt_ps = m5.tile([P, P], bf16, tag="rt_ps")
                    nc.tensor.transpose(rt_ps[:], mo_sb[:, hd, a * 128:(a + 1) * 128],
                                        ident_b_sb[:])
                    nc.vector.tensor_copy(ret_nat[:, a, hd * 128:(hd + 1) * 128], rt_ps[:])
            m5_cm.__exit__(None, None, None)
            if "ffn" in skip:
                for a in range(NST):
                    nc.vector.tensor_copy(ret_nat[:, a, :], zrow_sb[:, 0:H])
            nc.sync.dma_start(ret_in[:].rearrange("(a p) n -> p a n", p=P), ret_nat[:])

        if "coll" in skip:
            nc.sync.dma_start(ret_out[:], ret_in[:])
        else:
            nc.gpsimd.collective_compute(
                "AllToAll", AluOp.bypass, replica_groups=rg,
                ins=[ret_in.opt()], outs=[ret_out.opt()])

        # ================= combine + final =================
        _mark(nc, "final")
        with tc.tile_pool(name=r + "fin", bufs=2) as fw, \
             tc.tile_pool(name=r + "fps", bufs=2, space="PSUM") as fps:
            y_sb = []
            for tt in range(2):
                for j in range(2):
                    yk = fw.tile([P, H], bf16, tag=f"y{tt}{j}")
                    y_sb.append(yk)
                    if "front" in skip:
                        nc.vector.tensor_copy(yk[:], zrow_sb[:, 0:H])
                    else:
                        nc.gpsimd.indirect_dma_start(
                            out=yk[:], out_offset=None, in_=ret_out[:],
                            in_offset=bass.IndirectOffsetOnAxis(
                                ap=offc_sb[:, tt * 2 + j:tt * 2 + j + 1], axis=0),
                            bounds_check=SLOTS - 1, oob_is_err=False)
            if dbg:
                nc.sync.dma_start(dbg["d_y"][:, 0:H], y_sb[0][:])
                nc.sync.dma_start(dbg["d_y"][:, H:2 * H], y_sb[1][:])
            fin_sb = fw.tile([P, 2, H], f32, tag="fin_sb")
            for tt in range(2):
                y1w = fw.tile([P, H], f32, tag="y1w")
                nc.vector.tensor_scalar(y1w[:], y_sb[2 * tt][:],
                                        wc_sb[:, 2 * tt:2 * tt + 1], None, op0=AluOp.mult)
                y2w = fw.tile([P, H], f32, tag="y2w")
                nc.vector.tensor_scalar(y2w[:], y_sb[2 * tt + 1][:],
                                        wc_sb[:, 2 * tt + 1:2 * tt + 2], None, op0=AluOp.mult)
                ysum = fw.tile([P, H], f32, tag="ysum")
                nc.vector.tensor_add(ysum[:], y1w[:], y2w[:])
                for kt in range(8):
                    ht_ps = fps.tile([P, P], f32, tag="ht_ps")
                    nc.tensor.transpose(ht_ps[:], h2_sb[:, kt, tt * 128:(tt + 1) * 128],
                                        ident_sb[:])
                    nc.vector.tensor_add(fin_sb[:, tt, kt * 128:(kt + 1) * 128],
                                         ht_ps[:], ysum[:, kt * 128:(kt + 1) * 128])
            nc.sync.dma_start(t["out_blk"].rearrange("(a p) n -> p a n", p=P), fin_sb[:])


# ======================= host side =======================

def _rope_tables():
    pos = np.arange(S, dtype=np.float64)
    inv = 1.0 / (THETA ** (np.arange(0, HD, 2, dtype=np.float64) / HD))
    fr = pos[:, None] * inv[None, :]
    emb = np.concatenate([fr, fr], axis=-1)
    return np.cos(emb).astype(np.float32), np.sin(emb).astype(np.float32)


def _part_contig(mat):
    """[H, N] -> [P, 8*N]: row p holds [kt0 | kt1 | ...] chunks (kt*128+p)."""
    Hn, N = mat.shape
    kt = Hn // P
    return np.ascontiguousarray(
        mat.reshape(kt, P, N).transpose(1, 0, 2).reshape(P, kt * N))


def _prep_inputs(inputs):
    hs = np.asarray(inputs["hidden_states"], np.float32)[0]
    ln1 = np.asarray(inputs["ln1_w"], np.float32)
    ln2 = np.asarray(inputs["ln2_w"], np.float32)
    wq = np.asarray(inputs["wq"], np.float32) * ln1[None, :] / np.sqrt(HD)
    wk = np.asarray(inputs["wk"], np.float32) * ln1[None, :]
    wv = np.asarray(inputs["wv"], np.float32) * ln1[None, :]
    wo = np.asarray(inputs["wo"], np.float32)
    gate = np.asarray(inputs["gate_w"], np.float32) * ln2[None, :]
    w1 = np.asarray(inputs["w1"], np.float32) * ln2[None, None, :]
    w3 = np.asarray(inputs["w3"], np.float32) * ln2[None, None, :]
    w2 = np.asarray(inputs["w2"], np.float32)

    cos, sin = _rope_tables()
    hT = np.ascontiguousarray(hs.T)

    def rot_w(w_head):
        return np.concatenate([-w_head[32:64], w_head[0:32]], axis=0)

    ident = np.eye(P, dtype=np.float32)
    sidx = np.arange(P)
    tri_m = np.where(sidx[:, None] <= sidx[None, :], 0.0, NEG).astype(np.float32)
    onescol = np.ones((P, 1), np.float32)
    ones1 = np.ones((1, P), np.float32)
    mcomb = np.zeros((E, 28), np.float32)
    for ci, (a, b) in enumerate(COMBS):
        mcomb[a, ci] = 1.0
        mcomb[b, ci] = 1.0
    selmat = np.ascontiguousarray(mcomb.T)
    cumtri = np.triu(np.ones((E, E), np.float32))     # [e',e]=1 iff e'<=e
    ecapp = (np.arange(E, dtype=np.float32) * CAPP - 1.0).reshape(E, 1)
    zrow = np.zeros((P, 2048), BF16_NP)
    cosT_d = np.ascontiguousarray(np.concatenate([cos.T, cos.T], axis=0))
    sinT_d = np.ascontiguousarray(np.concatenate([sin.T, sin.T], axis=0))

    in_maps = []
    for c in range(NCORES):
        bA, bB = c, 15 - c
        tok = np.concatenate([np.arange(bA * P, bA * P + P),
                              np.arange(bB * P, bB * P + P)])
        qh0, qh1, kvh = 2 * c, 2 * c + 1, c
        wq0 = wq[qh0 * HD:(qh0 + 1) * HD]
        wq1 = wq[qh1 * HD:(qh1 + 1) * HD]
        wkc = wk[kvh * HD:(kvh + 1) * HD]
        wvc = wv[kvh * HD:(kvh + 1) * HD]
        wqkv = np.concatenate([
            wq0.T, wq1.T, rot_w(wq0).T, rot_w(wq1).T,
            wkc.T, rot_w(wkc).T, wvc.T], axis=1).astype(np.float32)
        # w13 contiguous: [NI, P, kt*256+j]; j<128 -> w1 row, j>=128 -> w3 row
        w13h = np.concatenate(
            [w1[c].reshape(NI, P, H).transpose(0, 2, 1),
             w3[c].reshape(NI, P, H).transpose(0, 2, 1)], axis=2)  # [NI, H, 256]
        w13c = w13h.reshape(NI, 8, P, 256).transpose(0, 2, 1, 3).reshape(NI, P, 2048)
        # w2 contiguous: [P, it*H + h] = w2[c][h, it*128+p]
        w2t = w2[c].T.reshape(NI, P, H)               # [it, p, h]
        w2c = w2t.transpose(1, 0, 2).reshape(P, NI * H)
        m = {
            "hT_full": hT,
            "hT_my": np.ascontiguousarray(hT[:, tok]),
            "wqkvT": np.ascontiguousarray(wqkv),
            "woT": _part_contig(np.ascontiguousarray(wo.T)).astype(BF16_NP),
            "gateT": _part_contig(np.ascontiguousarray(gate.T)),
            "cosT": cosT_d, "sinT": sinT_d,
            "ident": ident, "tri": tri_m,
            "onescol": onescol, "onescol_b": onescol.astype(BF16_NP),
            "onescol_r": onescol, "ones1_r": ones1,
            "epscol": np.full((P, 1), EPS, np.float32),
            "mcomb_r": mcomb, "selmat_r": selmat,
            "cumtri_r": cumtri, "ecapp": ecapp,
            "zrow": zrow,
            "w13": np.ascontiguousarray(w13c).astype(BF16_NP),
            "w2c": np.ascontiguousarray(w2c).astype(BF16_NP),
        }
        in_maps.append(m)
    return in_maps


_CACHE = {}


def _get_program(dbg=False, n_unroll=1):
    key = (dbg, n_unroll)
    if key not in _CACHE:
        _CACHE[key] = build_program(dbg=dbg, n_unroll=n_unroll)
    return _CACHE[key]


def run(inputs, dbg=False, n_unroll=1):
    nc = _get_program(dbg=dbg, n_unroll=n_unroll)
    in_maps = _prep_inputs(inputs)
    return bass_utils.run_bass_kernel_spmd(nc, in_maps, core_ids=list(range(NCORES)))


def assemble_output(res):
    out = np.zeros((S, H), np.float32)
    for c in range(NCORES):
        blk = res.results[c]["out_blk"]
        bA, bB = c, 15 - c
        out[bA * P:(bA + 1) * P] = blk[0:P]
        out[bB * P:(bB + 1) * P] = blk[P:256]
    return out.reshape(B, S, H)


def kernel(**inputs):
    res = run(inputs)
    return assemble_output(res)


# revision 5
# speedup vs baseline: 1.1610x; 1.1610x over previous
"""Mixtral decoder layer (GQA attention + top2-of-28-combination MoE) on 8 TRN2 cores.

v2 SPMD design (one program; per-core behavior injected via inputs):
  - Attention head-sharded: core c owns q-heads {2c,2c+1} / kv-head c over ALL
    tokens. Both q-heads share kv-head c, so scores/exp/ctx run on combined
    [keys=128, 2*256] tiles (half the instructions of v1). RoPE folded into a
    rotated-weight set. Softmax denominator via ones column appended to V.
    All activations bf16 (fp32 PSUM accumulation).
  - Context re-sharded token-wise via AllToAll; O-proj + residual + rmsnorm2 +
    router + top2-of-28 routing per 256-token zigzag block {c, 15-c} (f32).
  - MoE expert-parallel with A2A dispatch/combine: owner-local routing computes
    per-(expert, owner) slot positions (capacity CAPP); x2 rows scattered into
    a [E*CAPP, H] send buffer; AllToAll delivers each expert-core its tokens
    already compacted; FFN (bf16, fp32 accum, Silu); results transposed back to
    natural rows and AllToAll'd back; owners gather their two expert rows by
    the same offsets and combine with router weights + residual.
  - ln1/ln2 and 1/sqrt(HD) folded into weights host-side.
"""

import itertools

import numpy as np

import concourse.bass as bass
import concourse.tile as tile
from concourse import bacc, bass_utils, mybir

P = 128
B, S, H = 1, 2048, 1024
NH, KVH, HD = 16, 8, 64
E, TOPK, I = 8, 2, 3584
EPS = 1e-6
THETA = 1000000.0
NCORES = 8
NT = S // P
NPAIR = NT // 2
NI = I // P
CAPP = 96                   # per-(expert, owner) slot capacity
SLOTS = E * CAPP            # 896 rows through the dispatch/return A2As
NST = SLOTS // P            # 7 slot tiles
NEG = -1.0e30

f32 = mybir.dt.float32
f32r = mybir.dt.float32r
bf16 = mybir.dt.bfloat16
i32 = mybir.dt.int32
BF16_NP = mybir.dt.np(bf16)

COMBS = np.array(list(itertools.combinations(range(E), TOPK)), dtype=np.int64)

AluOp = mybir.AluOpType
Act = mybir.ActivationFunctionType
AxX = mybir.AxisListType.X

PHASE_MARKS = []


def _mark(nc, name):
    PHASE_MARKS.append((name, nc.next_id()))


def _z_a_of_block(b):
    return (b, 0) if b < 8 else (15 - b, 1)


def build_program(dbg: bool = False, n_unroll: int = 1, skip=()):
    """skip: subset of {"attn","ffn","coll","front"} for timing bisection."""
    nc = bacc.Bacc("TRN2", target_bir_lowering=False, debug=False,
                   num_devices=NCORES)

    def din(name, shape, dtype=f32):
        return nc.dram_tensor(name, list(shape), dtype, kind="ExternalInput").ap()

    t = {}
    t["hT_full"] = din("hT_full", [H, S])
    t["hT_my"] = din("hT_my", [H, 256])
    t["wqkvT"] = din("wqkvT", [H, 448])
    t["woT"] = din("woT", [P, 8 * H], bf16)          # [p, dt*H+j]
    t["gateT"] = din("gateT", [P, 8 * E])            # [p, kt*E+e]
    t["cosT"] = din("cosT", [P, S])
    t["sinT"] = din("sinT", [P, S])
    t["ident"] = din("ident", [P, P])
    t["tri"] = din("tri", [P, P])
    t["onescol"] = din("onescol", [P, 1])
    t["onescol_b"] = din("onescol_b", [P, 1], bf16)
    t["epscol"] = din("epscol", [P, 1])
    t["onescol_r"] = din("onescol_r", [P, 1], f32r)
    t["ones1_r"] = din("ones1_r", [1, P], f32r)
    t["mcomb_r"] = din("mcomb_r", [E, 28], f32r)
    t["selmat_r"] = din("selmat_r", [28, E], f32r)
    t["cumtri_r"] = din("cumtri_r", [E, E], f32r)    # [e',e]=1 iff e'<=e
    t["ecapp"] = din("ecapp", [E, 1])                # e*CAPP - 1
    t["zrow"] = din("zrow", [P, 2048], bf16)
    t["w13"] = din("w13", [NI, P, 2048], bf16)       # [it, p, kt*256+j]
    t["w2c"] = din("w2c", [P, NI * H], bf16)         # [p, it*H+h]

    def dout(name, shape, dtype=f32):
        return nc.dram_tensor(name, list(shape), dtype, kind="ExternalOutput").ap()

    t["out_blk"] = dout("out_blk", [256, H])
    t["dbg"] = {}
    if dbg:
        for nm, shp, dt_ in [
            ("d_x1T", [P, 512], f32), ("d_qr", [64, 1024], bf16),
            ("d_raw", [P, 8, 512], f32), ("d_ssq", [4, 512], f32),
            ("d_xn", [P, 8, 512], f32), ("d_qps", [P, 512], f32),
            ("d_kr", [64, S], bf16), ("d_ctxT", [P, S], bf16),
            ("d_h2T", [H, 256], f32), ("d_rt", [16, 256], f32),
            ("d_off", [P, 8], i32), ("d_xg", [P, SLOTS], bf16),
            ("d_hp", [P, SLOTS], bf16), ("d_y", [P, 2 * H], bf16),
        ]:
            t["dbg"][nm] = dout(nm, shp, dt_)

    PHASE_MARKS.clear()
    rg = [list(range(NCORES))]
    with tile.TileContext(nc) as tc:
        for rep in range(n_unroll):
            _emit_once(nc, tc, rg, t, rep, skip)
    nc.compile()
    return nc


def _emit_once(nc, tc, rg, t, rep, skip=()):
    _mark(nc, "start")
    dbg = t["dbg"] if rep == 0 else {}
    r = f"r{rep}_"

    with nc.allow_low_precision(reason="bf16 compute with f32 psum accum"), \
         tc.tile_pool(name=r + "const", bufs=1) as cpool, \
         tc.tile_pool(name=r + "big", bufs=1) as big, \
         tc.tile_pool(name=r + "dram", bufs=1, space="DRAM") as dram:

        _mark(nc, "consts")
        ident_sb = cpool.tile([P, P], f32)
        nc.sync.dma_start(ident_sb[:], t["ident"])
        ident_r_sb = cpool.tile([P, P], f32r)
        nc.sync.dma_start(ident_r_sb[:], t["ident"].bitcast(f32r))
        ident_b_sb = cpool.tile([P, P], bf16)
        nc.vector.tensor_copy(ident_b_sb[:], ident_sb[:])
        tri_sb = cpool.tile([P, P], f32)
        nc.sync.dma_start(tri_sb[:], t["tri"])
        onescol_sb = cpool.tile([P, 1], f32)
        nc.sync.dma_start(onescol_sb[:], t["onescol"])
        onescol_b_sb = cpool.tile([P, 1], bf16)
        nc.sync.dma_start(onescol_b_sb[:], t["onescol_b"])
        epscol_sb = cpool.tile([P, 1], f32)
        nc.sync.dma_start(epscol_sb[:], t["epscol"])
        onescol_r_sb = cpool.tile([P, 1], f32r)
        nc.sync.dma_start(onescol_r_sb[:], t["onescol_r"])
        ones1_r_sb = cpool.tile([1, P], f32r)
        nc.sync.dma_start(ones1_r_sb[:], t["ones1_r"])
        mcomb_sb = cpool.tile([E, 28], f32r)
        nc.sync.dma_start(mcomb_sb[:], t["mcomb_r"])
        selmat_sb = cpool.tile([28, E], f32r)
        nc.sync.dma_start(selmat_sb[:], t["selmat_r"])
        cumtri_sb = cpool.tile([E, E], f32r)
        nc.sync.dma_start(cumtri_sb[:], t["cumtri_r"])
        ecapp_sb = cpool.tile([E, 1], f32)
        nc.sync.dma_start(ecapp_sb[:], t["ecapp"])
        zrow_sb = cpool.tile([P, 2048], bf16)
        if skip:
            nc.sync.dma_start(zrow_sb[:], t["zrow"])

        _mark(nc, "load_h")
        hmy_sb = big.tile([P, 8, 256], f32)
        h2_sb = big.tile([P, 8, 256], f32)
        # dispatch offsets/weights, persist till combine: cols (tt*2+k)
        offc_sb = big.tile([P, 4], i32)
        wc_sb = big.tile([P, 4], f32)

        # ---- internal DRAM ----
        a2a_in = dram.tile([NCORES * P, 256], bf16)
        a2a_out = dram.tile([NCORES * P, 256], bf16)
        disp_in = dram.tile([SLOTS, H], bf16)
        disp_out = dram.tile([SLOTS, H], bf16)
        ret_in = dram.tile([SLOTS, H], bf16)
        ret_out = dram.tile([SLOTS, H], bf16)

        # ================= attention scope =================
        with tc.tile_pool(name=r + "attn", bufs=1) as apool, \
             tc.tile_pool(name=r + "aw", bufs=2) as aw:

            wqkv_sb = apool.tile([P, 8, 448], f32r)
            nc.sync.dma_start(wqkv_sb[:],
                              t["wqkvT"].bitcast(f32r).rearrange("(kt p) n -> p kt n", p=P))
            qr01_sb = apool.tile([64, NPAIR, 2, 256], bf16)
            kr_sb = apool.tile([64, S], bf16)
            vb_sb = apool.tile([P, NT, 65], bf16)
            a2a_sb = apool.tile([P, NCORES, 256], bf16)

            _mark(nc, "qkv_rope")
            rmsps_cm = tc.tile_pool(name=r + "rmsps", bufs=1, space="PSUM")
            rmsps = rmsps_cm.__enter__()
            qkvps_cm = tc.tile_pool(name=r + "qkvps", bufs=1, space="PSUM")
            qkvps = qkvps_cm.__enter__()
            qkvps2_cm = tc.tile_pool(name=r + "qkvps2", bufs=2, space="PSUM")
            qkvps2 = qkvps2_cm.__enter__()
            x1_sb = apool.tile([P, 8, S], f32r)    # normalized in place
            nc.sync.dma_start(x1_sb[:],
                              t["hT_full"].bitcast(f32r).rearrange("(kt p) n -> p kt n", p=P))
            cos_sb = apool.tile([P, S], f32)
            nc.sync.dma_start(cos_sb[:], t["cosT"])
            sin_sb = apool.tile([P, S], f32)
            nc.sync.dma_start(sin_sb[:], t["sinT"])
            for ntile in range(4):
                nsl = slice(ntile * 512, (ntile + 1) * 512)
                ssq = rmsps.tile([1, 512], f32, tag="ssq")
                for kt in range(8):
                    xsq = aw.tile([P, 512], f32r, tag="xsq")
                    nc.scalar.activation(xsq[:], x1_sb[:, kt, nsl], Act.Square)
                    nc.tensor.matmul(ssq[:], onescol_r_sb[:], xsq[:],
                                     start=(kt == 0), stop=(kt == 7))
                srow = aw.tile([1, 512], f32, tag="srow")
                nc.scalar.activation(srow[:], ssq[:], Act.Sqrt,
                                     bias=epscol_sb[0:1, :], scale=1.0 / H)
                srow_r = aw.tile([1, 512], f32r, tag="srow_r")
                nc.vector.reciprocal(srow_r[:], srow[:])
                sbc = rmsps.tile([P, 512], f32, tag="sbc")
                nc.tensor.matmul(sbc[:], ones1_r_sb[:1, :], srow_r[:],
                                 start=True, stop=True)
                for kt in range(8):
                    nc.vector.tensor_mul(x1_sb[:, kt, nsl], x1_sb[:, kt, nsl], sbc[:])
                # QKV + RoPE
                q_ps = qkvps.tile([P, 512], f32, tag="q_ps")
                qR_ps = qkvps.tile([P, 512], f32, tag="qR_ps")
                kk_ps = qkvps.tile([P, 512], f32, tag="kk_ps")
                v_ps = qkvps.tile([64, 512], f32, tag="v_ps")
                for kt in range(8):
                    x1s = x1_sb[:, kt, nsl]
                    st, sp = kt == 0, kt == 7
                    nc.tensor.matmul(q_ps[:], wqkv_sb[:, kt, 0:128], x1s, start=st, stop=sp)
                    nc.tensor.matmul(qR_ps[:], wqkv_sb[:, kt, 128:256], x1s, start=st, stop=sp)
                    nc.tensor.matmul(kk_ps[:], wqkv_sb[:, kt, 256:384], x1s, start=st, stop=sp)
                    nc.tensor.matmul(v_ps[:], wqkv_sb[:, kt, 384:448], x1s, start=st, stop=sp)
                t1 = aw.tile([P, 512], f32, tag="rope1")
                t2 = aw.tile([P, 512], f32, tag="rope2")
                nc.vector.tensor_mul(t1[:], q_ps[:], cos_sb[:, nsl])
                nc.vector.tensor_mul(t2[:], qR_ps[:], sin_sb[:, nsl])
                for j in range(2):
                    pr = 2 * ntile + j
                    csl = slice(j * 256, (j + 1) * 256)
                    nc.vector.tensor_add(qr01_sb[:, pr, 0, :], t1[0:64, csl], t2[0:64, csl])
                    nc.vector.tensor_add(qr01_sb[:, pr, 1, :], t1[64:128, csl], t2[64:128, csl])
                nc.vector.tensor_mul(t1[:64, :], kk_ps[0:64, :], cos_sb[0:64, nsl])
                nc.vector.tensor_mul(t2[:64, :], kk_ps[64:128, :], sin_sb[0:64, nsl])
                nc.vector.tensor_add(kr_sb[:, nsl], t1[:64, :], t2[:64, :])
                v_sb = aw.tile([64, 512], f32, tag="v_sb")
                nc.vector.tensor_copy(v_sb[:], v_ps[:])
                for tt in range(4):
                    ti = ntile * 4 + tt
                    vtp = qkvps2.tile([P, 64], f32, tag="vtp")
                    nc.tensor.transpose(vtp[:], v_sb[:, tt * 128:(tt + 1) * 128],
                                        ident_sb[:64, :64])
                    nc.vector.tensor_copy(vb_sb[:, ti, 0:64], vtp[:])
            nc.vector.tensor_copy(vb_sb[:, :, 64:65],
                                  onescol_b_sb[:].to_broadcast([P, NT, 1]))
            qkvps2_cm.__exit__(None, None, None)
            qkvps_cm.__exit__(None, None, None)
            rmsps_cm.__exit__(None, None, None)
            if dbg:
                nc.sync.dma_start(dbg["d_qr"],
                                  qr01_sb[:, 0:2, :, :].rearrange("p a b n -> p (a b n)"))
                nc.sync.dma_start(dbg["d_kr"], kr_sb[:])

            _mark(nc, "attn_core")
            attps_cm = tc.tile_pool(name=r + "attps", bufs=2, space="PSUM")
            attps = attps_cm.__enter__()
            attpsA_cm = tc.tile_pool(name=r + "attpsA", bufs=2, space="PSUM")
            attpsA = attpsA_cm.__enter__()
            for pr in ([] if "attn" in skip else range(NPAIR)):
                ctx = attpsA.tile([65, 512], f32, tag="ctx")
                for si in range(2 * pr + 2):
                    st_ps = attps.tile([P, 512], f32, tag="st_ps")
                    nc.tensor.matmul(st_ps[:], kr_sb[:, si * 128:(si + 1) * 128],
                                     qr01_sb[:, pr, :, :].rearrange("p a n -> p (a n)"),
                                     start=True, stop=True)
                    if si == 2 * pr:
                        nc.vector.tensor_add(st_ps[:, 0:128], st_ps[:, 0:128], tri_sb[:])
                        nc.vector.tensor_add(st_ps[:, 256:384], st_ps[:, 256:384], tri_sb[:])
                    if si == 2 * pr + 1:
                        nc.vector.tensor_add(st_ps[:, 128:256], st_ps[:, 128:256], tri_sb[:])
                        nc.vector.tensor_add(st_ps[:, 384:512], st_ps[:, 384:512], tri_sb[:])
                        nc.vector.tensor_scalar(st_ps[:, 0:128], st_ps[:, 0:128],
                                                NEG, None, op0=AluOp.add)
                        nc.vector.tensor_scalar(st_ps[:, 256:384], st_ps[:, 256:384],
                                                NEG, None, op0=AluOp.add)
                    stexp = aw.tile([P, 512], bf16, tag="stexp")
                    nc.scalar.activation(stexp[:], st_ps[:], Act.Exp)
                    nc.tensor.matmul(ctx[:], vb_sb[:, si, :], stexp[:],
                                     start=(si == 0), stop=(si == 2 * pr + 1))
                rec = aw.tile([1, 512], f32r, tag="rec")
                nc.vector.reciprocal(rec[:], ctx[64:65, :])
                dbc = attps.tile([64, 512], f32, tag="dbc")
                nc.tensor.matmul(dbc[:], ones1_r_sb[:1, 0:64], rec[:],
                                 start=True, stop=True)
                dbc_sb = aw.tile([64, 512], f32, tag="dbc_sb")
                nc.scalar.activation(dbc_sb[:], dbc[:], Act.Copy)
                for h in range(2):
                    for a in range(2):
                        ti = 2 * pr + a
                        z, az = _z_a_of_block(ti)
                        csl = slice(h * 256 + a * 128, h * 256 + a * 128 + 128)
                        nc.vector.tensor_mul(
                            a2a_sb[h * 64:h * 64 + 64, z, az * 128:az * 128 + 128],
                            ctx[0:64, csl], dbc_sb[:, csl])
            attpsA_cm.__exit__(None, None, None)
            attps_cm.__exit__(None, None, None)
            if "attn" in skip:
                nc.vector.tensor_copy(a2a_sb[:].rearrange("p c n -> p (c n)"),
                                      zrow_sb[:, 0:2048])
            _mark(nc, "a2a")
            nc.sync.dma_start(a2a_in[:].rearrange("(c p) n -> p c n", p=P), a2a_sb[:])
            if dbg:
                nc.sync.dma_start(dbg["d_ctxT"],
                                  a2a_sb[:].rearrange("p c n -> p (c n)"))

        if "coll" in skip:
            nc.sync.dma_start(a2a_out[:], a2a_in[:])
        else:
            nc.gpsimd.collective_compute(
                "AllToAll", AluOp.bypass, replica_groups=rg,
                ins=[a2a_in.opt()], outs=[a2a_out.opt()])

        # ================= O-proj + norm2 + router + dispatch =================
        nc.sync.dma_start(hmy_sb[:], t["hT_my"].rearrange("(kt p) n -> p kt n", p=P))
        _mark(nc, "oproj")
        with tc.tile_pool(name=r + "oproj", bufs=1) as opool, \
             tc.tile_pool(name=r + "ow", bufs=2) as ow:

            x2_sb = opool.tile([P, 8, 256], f32r)
            ctxmy_sb = opool.tile([P, 8, 256], bf16)
            nc.sync.dma_start(ctxmy_sb[:], a2a_out[:].rearrange("(c p) n -> p c n", p=P))
            if dbg:
                dcm = opool.tile([P, 8, 256], f32)
                for kt in range(8):
                    nc.vector.tensor_copy(dcm[:, kt, :], ctxmy_sb[:, kt, :])
                nc.sync.dma_start(dbg["d_raw"][:, :, 0:256], dcm[:])
            wo_sb = opool.tile([P, 8, H], bf16)
            nc.sync.dma_start(wo_sb[:], t["woT"].rearrange("p (dt n) -> p dt n", dt=8))
            o1_cm = tc.tile_pool(name=r + "o1", bufs=2, space="PSUM")
            o1 = o1_cm.__enter__()
            for hd in range(8):
                o_ps = o1.tile([P, 256], f32, tag="o_ps")
                for dt_ in range(8):
                    nc.tensor.matmul(o_ps[:], wo_sb[:, dt_, hd * 128:(hd + 1) * 128],
                                     ctxmy_sb[:, dt_, :], start=(dt_ == 0), stop=(dt_ == 7))
                nc.vector.tensor_add(h2_sb[:, hd, :], o_ps[:], hmy_sb[:, hd, :])
            if dbg:
                nc.sync.dma_start(dbg["d_h2T"].rearrange("(kt p) n -> p kt n", p=P), h2_sb[:])
            o1_cm.__exit__(None, None, None)

            _mark(nc, "rmsnorm2")
            o2_cm = tc.tile_pool(name=r + "o2", bufs=1, space="PSUM")
            o2 = o2_cm.__enter__()
            ssq2 = o2.tile([1, 256], f32, tag="ssq2")
            for kt in range(8):
                xsq2 = ow.tile([P, 256], f32r, tag="xsq2")
                nc.scalar.activation(xsq2[:], h2_sb[:, kt, :], Act.Square)
                nc.tensor.matmul(ssq2[:], onescol_r_sb[:], xsq2[:],
                                 start=(kt == 0), stop=(kt == 7))
            srow2 = ow.tile([1, 256], f32, tag="srow2")
            nc.scalar.activation(srow2[:], ssq2[:], Act.Sqrt,
                                 bias=epscol_sb[0:1, :], scale=1.0 / H)
            srow2_r = ow.tile([1, 256], f32r, tag="srow2_r")
            nc.vector.reciprocal(srow2_r[:], srow2[:])
            sbc2 = o2.tile([P, 256], f32, tag="sbc2")
            nc.tensor.matmul(sbc2[:], ones1_r_sb[:1, :], srow2_r[:], start=True, stop=True)
            for kt in range(8):
                nc.vector.tensor_mul(x2_sb[:, kt, :], h2_sb[:, kt, :], sbc2[:])
            o2_cm.__exit__(None, None, None)

            _mark(nc, "router")
            o3_cm = tc.tile_pool(name=r + "o3", bufs=1, space="PSUM")
            o3 = o3_cm.__enter__()
            gate_sb = opool.tile([P, 8, E], f32r)
            nc.sync.dma_start(gate_sb[:],
                              t["gateT"].bitcast(f32r).rearrange("p (kt n) -> p kt n", kt=8))
            rw_sb = ow.tile([P, 2, E], f32r, tag="rw")
            for tt in range(2):
                lg_ps = o3.tile([P, E], f32, tag="lg_ps")
                for kt in range(8):
                    nc.tensor.matmul(lg_ps[:], x2_sb[:, kt, tt * 128:(tt + 1) * 128],
                                     gate_sb[:, kt, :], start=(kt == 0), stop=(kt == 7))
                mx = ow.tile([P, 1], f32, tag="mx")
                nc.vector.tensor_reduce(mx[:], lg_ps[:], axis=AxX, op=AluOp.max)
                mxn = ow.tile([P, 1], f32, tag="mxn")
                nc.vector.tensor_scalar_mul(mxn[:], mx[:], -1.0)
                ex = ow.tile([P, E], f32, tag="ex")
                sm = ow.tile([P, 1], f32, tag="sm")
                nc.scalar.activation(ex[:], lg_ps[:], Act.Exp, bias=mxn[:], accum_out=sm[:])
                smr = ow.tile([P, 1], f32, tag="smr")
                nc.vector.reciprocal(smr[:], sm[:])
                nc.vector.tensor_scalar(rw_sb[:, tt, :], ex[:], smr[:], None, op0=AluOp.mult)
            rwT_sb = ow.tile([E, 256], f32r, tag="rwT")
            for tt in range(2):
                rwt_ps = o3.tile([E, P], f32r, tag="rwt_ps")
                nc.tensor.transpose(rwt_ps[:], rw_sb[:, tt, :], ident_r_sb[:])
                nc.vector.tensor_copy(rwT_sb[:, tt * 128:(tt + 1) * 128], rwt_ps[:])
            mask_sb = ow.tile([P, 2, 28], f32r, tag="mask")
            for tt in range(2):
                cb_ps = o3.tile([P, 28], f32, tag="cb_ps")
                nc.tensor.matmul(cb_ps[:], rwT_sb[:, tt * 128:(tt + 1) * 128],
                                 mcomb_sb[:], start=True, stop=True)
                mxc = ow.tile([P, 1], f32, tag="mxc")
                nc.vector.tensor_reduce(mxc[:], cb_ps[:], axis=AxX, op=AluOp.max)
                nc.vector.tensor_scalar(mask_sb[:, tt, :], cb_ps[:], mxc[:], None,
                                        op0=AluOp.is_ge)
            selT_sb = ow.tile([E, 256], f32r, tag="selT")
            for tt in range(2):
                mkt_ps = o3.tile([28, P], f32r, tag="mkt_ps")
                nc.tensor.transpose(mkt_ps[:], mask_sb[:, tt, :], ident_r_sb[:])
                mkt = ow.tile([28, P], f32r, tag="mkt")
                nc.vector.tensor_copy(mkt[:], mkt_ps[:])
                selq = o3.tile([E, P], f32, tag="selq")
                nc.tensor.matmul(selq[:], selmat_sb[:], mkt[:], start=True, stop=True)
                nc.vector.tensor_copy(selT_sb[:, tt * 128:(tt + 1) * 128], selq[:])
            # normalized top2 weights
            rwsel_sb = ow.tile([E, 256], f32r, tag="rwsel")
            nc.vector.tensor_mul(rwsel_sb[:], rwT_sb[:], selT_sb[:])
            nrm_ps = o3.tile([1, 256], f32, tag="nrm_ps")
            nc.tensor.matmul(nrm_ps[:], onescol_r_sb[:E, :], rwsel_sb[:],
                             start=True, stop=True)
            nrmr = ow.tile([1, 256], f32r, tag="nrmr")
            nc.vector.reciprocal(nrmr[:], nrm_ps[:])
            nbc_ps = o3.tile([E, 256], f32, tag="nbc_ps")
            nc.tensor.matmul(nbc_ps[:], ones1_r_sb[:1, :E], nrmr[:], start=True, stop=True)
            wsel_sb = ow.tile([E, 256], f32r, tag="wsel")
            nc.vector.tensor_mul(wsel_sb[:], rwsel_sb[:], nbc_ps[:])
            o3_cm.__exit__(None, None, None)

            _mark(nc, "routing")
            # first/second expert split + slot positions
            o5_cm = tc.tile_pool(name=r + "o5", bufs=1, space="PSUM")
            o5 = o5_cm.__enter__()
            cum_ps = o5.tile([E, 256], f32, tag="cum_ps")
            nc.tensor.matmul(cum_ps[:], cumtri_sb[:], selT_sb[:], start=True, stop=True)
            mle1 = ow.tile([E, 256], f32r, tag="mle1")
            nc.vector.tensor_scalar(mle1[:], cum_ps[:], 1.5, None, op0=AluOp.is_le)
            sel1 = ow.tile([E, 256], f32r, tag="sel1")
            nc.vector.tensor_mul(sel1[:], selT_sb[:], mle1[:])
            sel2 = ow.tile([E, 256], f32r, tag="sel2")
            nc.vector.tensor_sub(sel2[:], selT_sb[:], sel1[:])
            pos = ow.tile([E, 256], f32, tag="pos")
            nc.vector.tensor_tensor_scan(pos[:], selT_sb[:], selT_sb[:], 0.0,
                                         op0=AluOp.add, op1=AluOp.bypass)
            offa = ow.tile([E, 256], f32r, tag="offa")
            nc.vector.tensor_scalar(offa[:], pos[:], ecapp_sb[:], None, op0=AluOp.add)
            ovf = ow.tile([E, 256], f32r, tag="ovf")
            nc.vector.tensor_scalar(ovf[:], pos[:], CAPP + 0.5, None, op0=AluOp.is_ge)
            nc.vector.tensor_scalar(ovf[:], ovf[:], 8192.0, None, op0=AluOp.mult)
            nc.vector.tensor_add(offa[:], offa[:], ovf[:])
            # reduce to per-token rows: [1, 4*256] = [off1 | off2 | w1 | w2]
            tmp4 = ow.tile([E, 4, 256], f32r, tag="tmp4")
            for j, (selx, val) in enumerate(((sel1, offa), (sel2, offa),
                                             (sel1, wsel_sb), (sel2, wsel_sb))):
                nc.vector.tensor_mul(tmp4[:, j, :], selx[:], val[:])
            rows_ps = o5.tile([1, 4 * 256], f32, tag="rows_ps")
            tmp4f = tmp4[:].rearrange("p a n -> p (a n)")
            nc.tensor.matmul(rows_ps[:, 0:512], onescol_r_sb[:E, :],
                             tmp4f[:, 0:512], start=True, stop=True)
            nc.tensor.matmul(rows_ps[:, 512:1024], onescol_r_sb[:E, :],
                             tmp4f[:, 512:1024], start=True, stop=True)
            rows_sb = ow.tile([1, 4 * 256], f32, tag="rows_sb")
            nc.vector.tensor_copy(rows_sb[:], rows_ps[:])
            if dbg:
                nc.sync.dma_start(dbg["d_rt"][0:1, :],
                                  rows_sb[:, 0:256])
                nc.sync.dma_start(dbg["d_rt"][1:2, :], rows_sb[:, 256:512])
                nc.sync.dma_start(dbg["d_rt"][2:3, :], rows_sb[:, 512:768])
                nc.sync.dma_start(dbg["d_rt"][3:4, :], rows_sb[:, 768:1024])
            for tt in range(2):
                for j in range(2):  # off1, off2
                    cps = o5.tile([P, 1], f32, tag="cps")
                    nc.tensor.transpose(
                        cps[:], rows_sb[:, j * 256 + tt * 128:j * 256 + (tt + 1) * 128],
                        ident_sb[:1, :1])
                    nc.vector.tensor_copy(offc_sb[:, tt * 2 + j:tt * 2 + j + 1], cps[:])
                for j in range(2):  # w1, w2
                    cps = o5.tile([P, 1], f32, tag="cps")
                    nc.tensor.transpose(
                        cps[:], rows_sb[:, (2 + j) * 256 + tt * 128:(2 + j) * 256 + (tt + 1) * 128],
                        ident_sb[:1, :1])
                    nc.vector.tensor_copy(wc_sb[:, tt * 2 + j:tt * 2 + j + 1], cps[:])
            o5_cm.__exit__(None, None, None)

            _mark(nc, "x2nat")
            o4_cm = tc.tile_pool(name=r + "o4", bufs=2, space="PSUM")
            o4 = o4_cm.__enter__()
            x2n_sb = opool.tile([P, 2, H], bf16)
            for tt in range(2):
                for kt in range(8):
                    xt_ps = o4.tile([P, P], f32r, tag="xt_ps")
                    nc.tensor.transpose(xt_ps[:], x2_sb[:, kt, tt * 128:(tt + 1) * 128],
                                        ident_r_sb[:])
                    nc.vector.tensor_copy(x2n_sb[:, tt, kt * 128:(kt + 1) * 128], xt_ps[:])
            o4_cm.__exit__(None, None, None)

            _mark(nc, "dispatch")
            for tt in range(2):
                for j in ([] if "front" in skip else range(2)):
                    nc.gpsimd.indirect_dma_start(
                        out=disp_in[:],
                        out_offset=bass.IndirectOffsetOnAxis(
                            ap=offc_sb[:, tt * 2 + j:tt * 2 + j + 1], axis=0),
                        in_=x2n_sb[:, tt, :], in_offset=None,
                        bounds_check=SLOTS - 1, oob_is_err=False)
            if dbg:
                nc.sync.dma_start(dbg["d_off"][:, 0:4], offc_sb[:])
                nc.sync.dma_start(dbg["d_off"][:, 4:8], wc_sb[:].bitcast(i32))

        if "coll" in skip:
            nc.sync.dma_start(disp_out[:], disp_in[:])
        else:
            nc.gpsimd.collective_compute(
                "AllToAll", AluOp.bypass, replica_groups=rg,
                ins=[disp_in.opt()], outs=[disp_out.opt()])

        # ================= MoE FFN scope =================
        with tc.tile_pool(name=r + "moe", bufs=1) as mpool, \
             tc.tile_pool(name=r + "mw", bufs=2) as mw:

            _mark(nc, "gather")
            xg_sb = mpool.tile([P, 8, SLOTS], bf16)
            mx_cm = tc.tile_pool(name=r + "mx", bufs=1)
            mxp = mx_cm.__enter__()
            xin_sb = mxp.tile([P, NST, H], bf16)
            nc.sync.dma_start(xin_sb[:], disp_out[:].rearrange("(a p) n -> p a n", p=P))
            m2_cm = tc.tile_pool(name=r + "m2", bufs=2, space="PSUM")
            m2 = m2_cm.__enter__()
            for kt in range(8):
                for a in range(NST):
                    gt_ps = m2.tile([P, P], bf16, tag="gt_ps")
                    nc.tensor.transpose(gt_ps[:], xin_sb[:, a, kt * 128:(kt + 1) * 128],
                                        ident_b_sb[:])
                    nc.vector.tensor_copy(xg_sb[:, kt, a * 128:(a + 1) * 128], gt_ps[:])
            m2_cm.__exit__(None, None, None)
            mx_cm.__exit__(None, None, None)
            if dbg:
                nc.sync.dma_start(dbg["d_xg"], xg_sb[:, 0, :])

            _mark(nc, "ffnA")
            m3_cm = tc.tile_pool(name=r + "m3", bufs=2, space="PSUM")
            m3 = m3_cm.__enter__()
            hp_sb = mpool.tile([P, NI, SLOTS], bf16)
            for it in ([] if "ffn" in skip else range(NI)):
                w13_sb = mw.tile([P, 8, 256], bf16, tag="w13_sb")
                nc.sync.dma_start(w13_sb[:],
                                  t["w13"][it].rearrange("p (kt n) -> p kt n", kt=8))
                h1 = m3.tile([P, SLOTS], f32, tag="h1")
                h3 = m3.tile([P, SLOTS], f32, tag="h3")
                for kt in range(8):
                    st, sp = kt == 0, kt == 7
                    nc.tensor.matmul(h1[:, 0:512], w13_sb[:, kt, 0:128],
                                     xg_sb[:, kt, 0:512], start=st, stop=sp)
                    nc.tensor.matmul(h1[:, 512:768], w13_sb[:, kt, 0:128],
                                     xg_sb[:, kt, 512:768], start=st, stop=sp)
                    nc.tensor.matmul(h3[:, 0:512], w13_sb[:, kt, 128:256],
                                     xg_sb[:, kt, 0:512], start=st, stop=sp)
                    nc.tensor.matmul(h3[:, 512:768], w13_sb[:, kt, 128:256],
                                     xg_sb[:, kt, 512:768], start=st, stop=sp)
                sg = mw.tile([P, SLOTS], f32, tag="sg")
                nc.scalar.activation(sg[:], h1[:], Act.Sigmoid)
                hp1 = mw.tile([P, SLOTS], f32, tag="hp1")
                nc.vector.tensor_mul(hp1[:], h1[:], sg[:])
                nc.vector.tensor_mul(hp_sb[:, it, :], hp1[:], h3[:])
            m3_cm.__exit__(None, None, None)
            if dbg:
                nc.sync.dma_start(dbg["d_hp"], hp_sb[:, 0, :])

            _mark(nc, "ffnB")
            mo_sb = mpool.tile([P, 8, SLOTS], bf16)
            for cc, (c0, cw) in ([] if "ffn" in skip else list(enumerate(((0, 384), (384, 384))))):
                m4_cm = tc.tile_pool(name=r + f"m4_{cc}", bufs=1, space="PSUM")
                m4 = m4_cm.__enter__()
                mo_ps = []
                for hd in range(8):
                    mo_hd = m4.tile([P, cw], f32, tag=f"mo{hd}")
                    mo_ps.append(mo_hd)
                for it in range(NI):
                    w2_sb = mw.tile([P, H], bf16, tag="w2_sb")
                    nc.sync.dma_start(
                        w2_sb[:],
                        t["w2c"].rearrange("p (it n) -> p it n", it=NI)[:, it, :])
                    for hd in range(8):
                        nc.tensor.matmul(mo_ps[hd][:], w2_sb[:, hd * 128:(hd + 1) * 128],
                                         hp_sb[:, it, c0:c0 + cw],
                                         start=(it == 0), stop=(it == NI - 1))
                for hd in range(8):
                    nc.vector.tensor_copy(mo_sb[:, hd, c0:c0 + cw], mo_ps[hd][:])
                m4_cm.__exit__(None, None, None)

            _mark(nc, "retT")
            ret_nat = mpool.tile([P, NST, H], bf16)
            m5_cm = tc.tile_pool(name=r + "m5", bufs=2, space="PSUM")
            m5 = m5_cm.__enter__()
            for a in ([] if "ffn" in skip else range(NST)):
                for hd in range(8):
                    rt_ps = m5.tile([P, P], bf16, tag="rt_ps")
                    nc.tensor.transpose(rt_ps[:], mo_sb[:, hd, a * 128:(a + 1) * 128],
                                        ident_b_sb[:])
                    nc.vector.tensor_copy(ret_nat[:, a, hd * 128:(hd + 1) * 128], rt_ps[:])
            m5_cm.__exit__(None, None, None)
            if "ffn" in skip:
                for a in range(NST):
                    nc.vector.tensor_copy(ret_nat[:, a, :], zrow_sb[:, 0:H])
            nc.sync.dma_start(ret_in[:].rearrange("(a p) n -> p a n", p=P), ret_nat[:])

        if "coll" in skip:
            nc.sync.dma_start(ret_out[:], ret_in[:])
        else:
            nc.gpsimd.collective_compute(
                "AllToAll", AluOp.bypass, replica_groups=rg,
                ins=[ret_in.opt()], outs=[ret_out.opt()])

        # ================= combine + final =================
        _mark(nc, "final")
        with tc.tile_pool(name=r + "fin", bufs=2) as fw, \
             tc.tile_pool(name=r + "fps", bufs=2, space="PSUM") as fps:
            y_sb = []
            for tt in range(2):
                for j in range(2):
                    yk = fw.tile([P, H], bf16, tag=f"y{tt}{j}")
                    y_sb.append(yk)
                    if "front" in skip:
                        nc.vector.tensor_copy(yk[:], zrow_sb[:, 0:H])
                    else:
                        nc.gpsimd.indirect_dma_start(
                            out=yk[:], out_offset=None, in_=ret_out[:],
                            in_offset=bass.IndirectOffsetOnAxis(
                                ap=offc_sb[:, tt * 2 + j:tt * 2 + j + 1], axis=0),
                            bounds_check=SLOTS - 1, oob_is_err=False)
            if dbg:
                nc.sync.dma_start(dbg["d_y"][:, 0:H], y_sb[0][:])
                nc.sync.dma_start(dbg["d_y"][:, H:2 * H], y_sb[1][:])
            fin_sb = fw.tile([P, 2, H], f32, tag="fin_sb")
            for tt in range(2):
                y1w = fw.tile([P, H], f32, tag="y1w")
                nc.vector.tensor_scalar(y1w[:], y_sb[2 * tt][:],
                                        wc_sb[:, 2 * tt:2 * tt + 1], None, op0=AluOp.mult)
                y2w = fw.tile([P, H], f32, tag="y2w")
                nc.vector.tensor_scalar(y2w[:], y_sb[2 * tt + 1][:],
                                        wc_sb[:, 2 * tt + 1:2 * tt + 2], None, op0=AluOp.mult)
                ysum = fw.tile([P, H], f32, tag="ysum")
                nc.vector.tensor_add(ysum[:], y1w[:], y2w[:])
                for kt in range(8):
                    ht_ps = fps.tile([P, P], f32, tag="ht_ps")
                    nc.tensor.transpose(ht_ps[:], h2_sb[:, kt, tt * 128:(tt + 1) * 128],
                                        ident_sb[:])
                    nc.vector.tensor_add(fin_sb[:, tt, kt * 128:(kt + 1) * 128],
                                         ht_ps[:], ysum[:, kt * 128:(kt + 1) * 128])
            nc.sync.dma_start(t["out_blk"].rearrange("(a p) n -> p a n", p=P), fin_sb[:])


# ======================= host side =======================

def _rope_tables():
    pos = np.arange(S, dtype=np.float64)
    inv = 1.0 / (THETA ** (np.arange(0, HD, 2, dtype=np.float64) / HD))
    fr = pos[:, None] * inv[None, :]
    emb = np.concatenate([fr, fr], axis=-1)
    return np.cos(emb).astype(np.float32), np.sin(emb).astype(np.float32)


def _part_contig(mat):
    """[H, N] -> [P, 8*N]: row p holds [kt0 | kt1 | ...] chunks (kt*128+p)."""
    Hn, N = mat.shape
    kt = Hn // P
    return np.ascontiguousarray(
        mat.reshape(kt, P, N).transpose(1, 0, 2).reshape(P, kt * N))


def _prep_inputs(inputs):
    hs = np.asarray(inputs["hidden_states"], np.float32)[0]
    ln1 = np.asarray(inputs["ln1_w"], np.float32)
    ln2 = np.asarray(inputs["ln2_w"], np.float32)
    wq = np.asarray(inputs["wq"], np.float32) * ln1[None, :] / np.sqrt(HD)
    wk = np.asarray(inputs["wk"], np.float32) * ln1[None, :]
    wv = np.asarray(inputs["wv"], np.float32) * ln1[None, :]
    wo = np.asarray(inputs["wo"], np.float32)
    gate = np.asarray(inputs["gate_w"], np.float32) * ln2[None, :]
    w1 = np.asarray(inputs["w1"], np.float32) * ln2[None, None, :]
    w3 = np.asarray(inputs["w3"], np.float32) * ln2[None, None, :]
    w2 = np.asarray(inputs["w2"], np.float32)

    cos, sin = _rope_tables()
    hT = np.ascontiguousarray(hs.T)

    def rot_w(w_head):
        return np.concatenate([-w_head[32:64], w_head[0:32]], axis=0)

    ident = np.eye(P, dtype=np.float32)
    sidx = np.arange(P)
    tri_m = np.where(sidx[:, None] <= sidx[None, :], 0.0, NEG).astype(np.float32)
    onescol = np.ones((P, 1), np.float32)
    ones1 = np.ones((1, P), np.float32)
    mcomb = np.zeros((E, 28), np.float32)
    for ci, (a, b) in enumerate(COMBS):
        mcomb[a, ci] = 1.0
        mcomb[b, ci] = 1.0
    selmat = np.ascontiguousarray(mcomb.T)
    cumtri = np.triu(np.ones((E, E), np.float32))     # [e',e]=1 iff e'<=e
    ecapp = (np.arange(E, dtype=np.float32) * CAPP - 1.0).reshape(E, 1)
    zrow = np.zeros((P, 2048), BF16_NP)
    cosT_d = np.ascontiguousarray(np.concatenate([cos.T, cos.T], axis=0))
    sinT_d = np.ascontiguousarray(np.concatenate([sin.T, sin.T], axis=0))

    in_maps = []
    for c in range(NCORES):
        bA, bB = c, 15 - c
        tok = np.concatenate([np.arange(bA * P, bA * P + P),
                              np.arange(bB * P, bB * P + P)])
        qh0, qh1, kvh = 2 * c, 2 * c + 1, c
        wq0 = wq[qh0 * HD:(qh0 + 1) * HD]
        wq1 = wq[qh1 * HD:(qh1 + 1) * HD]
        wkc = wk[kvh * HD:(kvh + 1) * HD]
        wvc = wv[kvh * HD:(kvh + 1) * HD]
        wqkv = np.concatenate([
            wq0.T, wq1.T, rot_w(wq0).T, rot_w(wq1).T,
            wkc.T, rot_w(wkc).T, wvc.T], axis=1).astype(np.float32)
        # w13 contiguous: [NI, P, kt*256+j]; j<128 -> w1 row, j>=128 -> w3 row
        w13h = np.concatenate(
            [w1[c].reshape(NI, P, H).transpose(0, 2, 1),
             w3[c].reshape(NI, P, H).transpose(0, 2, 1)], axis=2)  # [NI, H, 256]
        w13c = w13h.reshape(NI, 8, P, 256).transpose(0, 2, 1, 3).reshape(NI, P, 2048)
        # w2 contiguous: [P, it*H + h] = w2[c][h, it*128+p]
        w2t = w2[c].T.reshape(NI, P, H)               # [it, p, h]
        w2c = w2t.transpose(1, 0, 2).reshape(P, NI * H)
        m = {
            "hT_full": hT,
            "hT_my": np.ascontiguousarray(hT[:, tok]),
            "wqkvT": np.ascontiguousarray(wqkv),
            "woT": _part_contig(np.ascontiguousarray(wo.T)).astype(BF16_NP),
            "gateT": _part_contig(np.ascontiguousarray(gate.T)),
            "cosT": cosT_d, "sinT": sinT_d,
            "ident": ident, "tri": tri_m,
            "onescol": onescol, "onescol_b": onescol.astype(BF16_NP),
            "onescol_r": onescol, "ones1_r": ones1,
            "epscol": np.full((P, 1), EPS, np.float32),
            "mcomb_r": mcomb, "selmat_r": selmat,
            "cumtri_r": cumtri, "ecapp": ecapp,
            "zrow": zrow,
            "w13": np.ascontiguousarray(w13c).astype(BF16_NP),
            "w2c": np.ascontiguousarray(w2c).astype(BF16_NP),
        }
        in_maps.append(m)
    return in_maps


_CACHE = {}


def _get_program(dbg=False, n_unroll=1):
    key = (dbg, n_unroll)
    if key not in _CACHE:
        _CACHE[key] = build_program(dbg=dbg, n_unroll=n_unroll)
    return _CACHE[key]


def run(inputs, dbg=False, n_unroll=1):
    nc = _get_program(dbg=dbg, n_unroll=n_unroll)
    in_maps = _prep_inputs(inputs)
    return bass_utils.run_bass_kernel_spmd(nc, in_maps, core_ids=list(range(NCORES)))


def assemble_output(res):
    out = np.zeros((S, H), np.float32)
    for c in range(NCORES):
        blk = res.results[c]["out_blk"]
        bA, bB = c, 15 - c
        out[bA * P:(bA + 1) * P] = blk[0:P]
        out[bB * P:(bB + 1) * P] = blk[P:256]
    return out.reshape(B, S, H)


def kernel(**inputs):
    res = run(inputs)
    return assemble_output(res)
